# revision 18
# baseline (speedup 1.0000x reference)
"""DispMVS depth-fusion kernel for 8 Trainium2 NeuronCores.

Sharding: core c handles batch b = c // 4 and coarse rows r0 = (c % 4) * 64
(64 of 256 rows), with BOTH neighbor streams (NN=2) resident on the core
(partitions = nn*64 + row).  The cross-neighbor confidence-fusion softmax is
then core-local; cores never communicate.

Pipeline per core (one Bass/Tile program, identical for all 8 cores):
  1. prerun: mask DMAs + exp(mask) for the first two chunks are emitted ahead
     of geometry so the ACT engine works while the DVE runs the geometry chain
     (separate tile pools avoid the SBUF-reuse barrier).
  2. geometry: elementwise epipolar math -> clipped inverse depth (fp32 math,
     fp16 result) on DVE+ACT; denominators here are provably bounded away
     from 0, so the reference's abs/eps guards are algebraically dropped.
  3. on-chip halo build: SBUF->SBUF DMAs create 3 vertically shifted rows of
     inv-depth (t3i); conf rows come host-padded (t3c).  Shifted copies build
     the per-tap operand planes ufi/ufc [128, 9, 320] fp16.
  4. chunk loop (4 row-subpixel groups x 2 w-halves), all mask-sized work in
     fp16 with k-major planes [128, 9k, 4q, 160w]:
       ACT  exp(mask), tanh-based 2-view softmax weight, output affine
       DVE  e*ufi, e*ufc (fp16 TT at 2x), joint 3-tag 9-sum tree,
            reciprocals, weighted avgs, fusion TTs
     Tails (final recip + output DMA) are deferred one chunk so cross-engine
     latency never stalls the DVE queue.
"""

import numpy as np

NN, B, H, W = 2, 2, 256, 320
UP = 4
EPS = 1e-6
RPC = 64          # coarse rows per core
NCORES = 8
WC = 160          # w-half width
KQ = 9 * 4 * WC   # elements per chunk per partition (5760)
PL = 4 * WC       # elements per k-plane (640)

# consts columns
(
    C_M00, C_M01, C_M02, C_M10, C_M11, C_M12, C_M20, C_M21, C_M22,
    C_T0, C_T1, C_T2,
    C_R00, C_R01, C_R02, C_R10, C_R11, C_R12, C_R20, C_R21, C_R22,
    C_A0, C_A1, C_A2, C_B0, C_B1, C_B2,
    C_TX, C_TY, C_TZ,
    C_CA, C_CB, C_DS2, C_DB, C_TEN,
) = range(35)
NCONST = 36

_cache = {}


def _register_custom_ops():
    """Register this kernel's custom DVE ops (idempotent). Returns a dict.

    SUMSQ_ANT: out = in0^2 + in1^2 (one pass instead of 3).
    RSQRT_NR_ANT: one Newton step for 1/sqrt: out = in0*(1.5 - 0.5*in1*in0^2)
      (one pass instead of 4).
    MUL_CUMSUM_ANT kept for sub-opcode stability with earlier builds.
    """
    from concourse import dve_ops
    from concourse.dve_spec import AluOp, C0, C1, Spec, Src0, Src1, _has_src1, lower, scan
    from concourse.dve_uop import DveOpSpec

    have = {o.name: o for o in dve_ops.OPS}
    if "MUL_CUMSUM_ANT" in have:
        return have

    def cum_ref(in0, in1, s0, s1, imm2):
        a = in0.astype(np.float32).reshape(in0.shape[0], -1) * in1.astype(
            np.float32
        ).reshape(in1.shape[0], -1)
        return np.cumsum(a, axis=1, dtype=np.float32).reshape(in0.shape)

    specs = [
        ("MUL_CUMSUM_ANT", Spec(body=scan(AluOp.ADD, Src0 * Src1), reference=cum_ref)),
        (
            "SUMSQ_ANT",
            Spec(
                body=Src0 * Src0 + Src1 * Src1,
                reference=lambda in0, in1, s0, s1, imm2: (
                    in0.astype(np.float32) ** 2 + in1.astype(np.float32) ** 2
                ),
            ),
        ),
        (
            "RSQRT_NR_ANT",
            Spec(
                body=(Src0 * Src0 * Src1 * C0 + C1) * Src0,
                reference=lambda in0, in1, s0, s1, imm2: (
                    (in0.astype(np.float32) ** 2 * in1 * s0 + s1) * in0
                ),
            ),
        ),
    ]
    out = {}
    for name, spec in specs:
        op_ = dve_ops.DveOp(name, spec, subdim=False, uops_sha={})
        dve_ops.OPS.append(op_)
        dve_ops.CUSTOM_DVE_SPECS[name] = spec
        dve_ops._SUB_OPCODE_FOR_NAME[name] = (
            dve_ops._CUSTOM_DVE_ROW_BASE + len(dve_ops.OPS) - 1
        )
        for ver in ("v3", "v4"):
            tmp = DveOpSpec(
                name=name,
                opcode=dve_ops.get_dve_sub_opcode(name),
                uops=lower(spec, ver=ver),
                rd1_en=_has_src1(spec),
            )
            op_.uops_sha[ver] = tmp.sha(ver)
        out[name] = op_
    assert max(dve_ops._SUB_OPCODE_FOR_NAME.values()) < 0x20
    return out


def _build_program():
    import concourse.bass as bass
    import concourse.bacc as bacc
    import concourse.tile as tile
    from concourse import mybir
    from concourse.alu_op_type import AluOpType as op

    f32 = mybir.dt.float32
    f16 = mybir.dt.float16
    i32 = mybir.dt.int32
    Act = mybir.ActivationFunctionType

    cops = _register_custom_ops()

    # This kernel alternates Exp and Ln activations; the greedy table-load
    # placement would thrash ACT_TABLE_LOADs between the exp-only and ln-only
    # tables (~1.3us each).  Blank every other table so the joint
    # natural_log_exp_and_others set (which has Exp/Ln/Identity/Copy/Abs)
    # serves everything with a single load.  Order is preserved so the
    # act_func_set_id indices stay valid.
    import concourse.hw_specs as hw_specs

    if not getattr(bacc, "_ant_act_tables_patched", False):
        _orig_gat = hw_specs.get_activation_tables

        def _gat(module_arch):
            tabs = _orig_gat(module_arch)
            if "natural_log_exp_and_others" in tabs:
                joint = tabs["natural_log_exp_and_others"]
                tabs = {
                    name: (funcs if name == "natural_log_exp_and_others"
                           else funcs - joint)
                    for name, funcs in tabs.items()
                }
            return tabs

        hw_specs.get_activation_tables = _gat
        bacc.get_activation_tables = _gat
        bacc._ant_act_tables_patched = True

    nc = bacc.Bacc("TRN2", target_bir_lowering=False, debug=False)

    pix_d = nc.dram_tensor("pix", [128, 4, 330], f32, kind="ExternalInput").ap()
    consts_d = nc.dram_tensor("consts", [128, NCONST], f32, kind="ExternalInput").ap()
    hm_d = nc.dram_tensor("hm", [128, 10], f16, kind="ExternalInput").ap()
    confpad_d = nc.dram_tensor("confpad", [NN, 66, 322], f16, kind="ExternalInput").ap()
    # mask pre-packed on host to [gc, wc, (nn,r), k, q2, w] fp16 (k-major planes)
    mask_d = nc.dram_tensor("maskpk", [4, 2, 128, KQ], f16, kind="ExternalInput").ap()
    out_d = nc.dram_tensor("out", [RPC * UP, W * UP], f32, kind="ExternalOutput").ap()

    def dram_ap(base, off, dims):
        return bass.AP(tensor=base.tensor, offset=base.offset + off, ap=[list(d) for d in dims])

    def sap(a, off, dims):
        """Manual SBUF AP: keep the tile's partition dim, custom free dims."""
        return bass.AP(
            tensor=a.tensor, offset=a.offset + off,
            ap=[list(a.ap[0])] + [list(d) for d in dims],
        )

    CHUNKS = [(w_, g_) for w_ in range(2) for g_ in range(4)]

    with tile.TileContext(nc) as tc:
        with tc.tile_pool(name="persist", bufs=1) as pp, tc.tile_pool(
            name="io", bufs=2
        ) as io:
            consts = pp.tile([128, NCONST], f32, name="consts")
            nc.sync.dma_start(out=consts[:], in_=consts_d)

            def CC(i, p0=0, p1=128):
                return consts[p0:p1, i : i + 1]

            pix = pp.tile([128, 4, 330], f32, name="pix")
            nc.sync.dma_start(out=pix[:], in_=pix_d)

            hm = pp.tile([128, 10], f16, name="hm")
            t3i = pp.tile([128, 3, 322], f16, name="t3i")
            t3c = pp.tile([128, 3, 322], f16, name="t3c")
            ufi = pp.tile([128, 9, 320], f16, name="ufi")
            ufc = pp.tile([128, 9, 320], f16, name="ufc")
            inv16 = pp.tile([128, 330], f16, name="inv16")

            # ---- prerun: mask DMA + exp for the first two chunks ----
            def emit_head(ci, bufs=2):
                gc, wc = CHUNKS[ci][1], CHUNKS[ci][0]
                M = io.tile([128, 9, PL], f16, name="m", tag="m")
                nc.sync.dma_start(out=M[:], in_=mask_d[gc, wc])
                EM = io.tile([128, 27, PL], f16, name="em", tag="em", bufs=3)
                nc.scalar.activation(out=EM[:, 0:9, :], in_=M[:], func=Act.Exp)
                return EM

            pre = [emit_head(0), emit_head(1), emit_head(2)]

            nc.sync.dma_start(out=hm[:], in_=hm_d)
            # conf unfold rows straight from the host-padded input
            for nn in range(NN):
                src = dram_ap(
                    confpad_d, nn * 66 * 322,
                    [[322, 64], [322, 3], [1, 322]],
                )
                nc.sync.dma_start(out=t3c[nn * 64 : nn * 64 + 64], in_=src)
            # build conf tap planes early (independent of geometry)
            for dy in range(3):
                nc.scalar.activation(
                    out=sap(ufc[:], 3 * dy * 320, [[320, 3], [1, 320]]),
                    in_=sap(t3c[:], dy * 322, [[1, 3], [1, 320]]),
                    func=Act.Copy,
                )
            # zero t3i (cols 0/321 stay 0; rows overwritten below)
            nc.vector.memset(t3i[:], 0.0)

            # ---- prerun: mask DMA + exp for the first two chunks ----
            def emit_head(ci, bufs=2):
                gc, wc = CHUNKS[ci][1], CHUNKS[ci][0]
                M = io.tile([128, 9, PL], f16, name="m", tag="m")
                nc.sync.dma_start(out=M[:], in_=mask_d[gc, wc])
                EM = io.tile([128, 27, PL], f16, name="em", tag="em", bufs=3)
                nc.scalar.activation(out=EM[:, 0:9, :], in_=M[:], func=Act.Exp)
                return EM

            pre = [emit_head(0), emit_head(1), emit_head(2)]

            nc.sync.dma_start(out=hm[:], in_=hm_d)
            # conf unfold rows straight from the host-padded input            # ---------------- geometry (fp32, DVE+ACT) ----------------
            u = pix[:, 0, :]
            v = pix[:, 1, :]
            d = pix[:, 2, :]
            fl = pix[:, 3, :]

            with tc.tile_pool(name="geom", bufs=1) as gp:
                _tagn = [0]

                def T():
                    _tagn[0] += 1
                    return gp.tile([128, 330], f32, name=f"g{_tagn[0]}", tag=f"g{_tagn[0]}")

                V = nc.vector

                def TT(o, a, b, alu):
                    V.tensor_tensor(out=o, in0=a, in1=b, op=alu)

                def TS(o, a, s1, o0, s2=None, o1=None):
                    if o1 is None:
                        V.tensor_scalar(out=o, in0=a, scalar1=s1, scalar2=None, op0=o0)
                    else:
                        V.tensor_scalar(out=o, in0=a, scalar1=s1, scalar2=s2, op0=o0, op1=o1)

                def STT(o, a, s, b, o0, o1):
                    V.scalar_tensor_tensor(out=o, in0=a, scalar=s, in1=b, op0=o0, op1=o1)

                def AF(o, a, scale, bias, func=Act.Identity):
                    nc.scalar.activation(out=o, in_=a, func=func, scale=scale, bias=bias)

                def AB(o, a):
                    nc.scalar.activation(out=o, in_=a, func=Act.Abs)

                def RF(o, x):
                    V.reciprocal_approx_fast(out=o, in_=x)

                # linear forms of (u, v): a_j = (K@R) @ [u,v,1], r_j = R @ [u,v,1]
                # (tensor_scalar on DVE: fp32 SBUF runs the fast 2x path and
                # keeps the chain off the exp-loaded ACT engine)
                a0, a1 = T(), T()
                rx, ry, rz = T(), T(), T()
                t1_, t2_, t3_, t4_, t5_, t6_, t7_ = T(), T(), T(), T(), T(), T(), T()
                TS(t2_[:], u, CC(C_M00), op.mult, CC(C_M02), op.add)
                STT(a0[:], v, CC(C_M01), t2_[:], op.mult, op.add)
                TS(t3_[:], u, CC(C_M10), op.mult, CC(C_M12), op.add)
                STT(a1[:], v, CC(C_M11), t3_[:], op.mult, op.add)
                TS(t4_[:], u, CC(C_R20), op.mult, CC(C_R22), op.add)
                STT(rz[:], v, CC(C_R21), t4_[:], op.mult, op.add)
                a2 = rz  # K row 2 is [0,0,1], so (K@R) row 2 == R row 2
                TS(t5_[:], u, CC(C_R00), op.mult, CC(C_R02), op.add)
                STT(rx[:], v, CC(C_R01), t5_[:], op.mult, op.add)
                TS(t6_[:], u, CC(C_R10), op.mult, CC(C_R12), op.add)
                STT(ry[:], v, CC(C_R11), t6_[:], op.mult, op.add)
                d10 = T()
                TS(d10[:], d, CC(C_TEN), op.add)

                # z components (always positive here) and reciprocals
                ps2, pe2, rs2, re2, m1, m2 = T(), T(), T(), T(), T(), T()
                TT(m1[:], a2[:], d, op.mult)
                TS(ps2[:], m1[:], CC(C_T2), op.add)
                RF(rs2[:], ps2[:])
                TT(m2[:], a2[:], d10[:], op.mult)
                TT(pe2[:], m2[:], ps2[:], op.add)
                RF(re2[:], pe2[:])

                # x/y projections (start and end)
                pxs, pxe, pys, pye = T(), T(), T(), T()
                psx, psy = T(), T()
                TT(t1_[:], a0[:], d, op.mult)
                TS(psx[:], t1_[:], CC(C_T0), op.add)
                TT(pxs[:], psx[:], rs2[:], op.mult)
                TT(t2_[:], a0[:], d10[:], op.mult)
                TT(t2_[:], t2_[:], psx[:], op.add)
                TT(pxe[:], t2_[:], re2[:], op.mult)
                TT(t3_[:], a1[:], d, op.mult)
                TS(psy[:], t3_[:], CC(C_T1), op.add)
                TT(pys[:], psy[:], rs2[:], op.mult)
                TT(t4_[:], a1[:], d10[:], op.mult)
                TT(t4_[:], t4_[:], psy[:], op.add)
                TT(pye[:], t4_[:], re2[:], op.mult)

                fdx, fdy = T(), T()
                TT(fdx[:], pxe[:], pxs[:], op.subtract)
                TT(fdy[:], pye[:], pys[:], op.subtract)

                # fl/sqrt(fdx^2+fdy^2) via magic seed + 2 fused Newton steps
                q, y, y2 = T(), T(), T()
                V._custom_dve(cops["SUMSQ_ANT"], out=q[:], in0=fdx[:], in1=fdy[:])
                yi = y[:].bitcast(i32)
                TS(yi, q[:].bitcast(i32), 1, op.arith_shift_right)
                TS(yi, yi, -1, op.bitwise_xor)
                TS(yi, yi, 0x5F3759DF + 1, op.add)
                V._custom_dve(cops["RSQRT_NR_ANT"], out=y2[:], in0=y[:], in1=q[:], s0=-0.5, s1=1.5)
                V._custom_dve(cops["RSQRT_NR_ANT"], out=y[:], in0=y2[:], in1=q[:], s0=-0.5, s1=1.5)

                fls, mx, my = d10, psx, psy  # dead by now; reuse
                TT(fls[:], fl, y[:], op.mult)
                TT(t5_[:], fdx[:], fls[:], op.mult)
                TT(mx[:], t5_[:], pxs[:], op.add)
                TT(t6_[:], fdy[:], fls[:], op.mult)
                TT(my[:], t6_[:], pys[:], op.add)

                ax, fm = T(), T()
                fmi = fm[:].bitcast(i32)
                TS(ax[:].bitcast(i32), fdx[:].bitcast(i32), 0x7FFFFFFF, op.bitwise_and)
                TS(t7_[:].bitcast(i32), fdy[:].bitcast(i32), 0x7FFFFFFF, op.bitwise_and)
                TT(fmi, t7_[:], ax[:], op.is_gt)

                nx, ny = pxe, pye  # dead after fdx/fdy; reuse
                TS(t1_[:], mx[:], CC(C_A0), op.mult, CC(C_A2), op.add)
                STT(nx[:], my[:], CC(C_A1), t1_[:], op.mult, op.add)
                TS(t2_[:], mx[:], CC(C_B0), op.mult, CC(C_B2), op.add)
                STT(ny[:], my[:], CC(C_B1), t2_[:], op.mult, op.add)

                def inv_axis(o, nj, rj, c_t, s1, s2, s3):
                    TT(s1[:], rz[:], nj[:], op.mult)
                    TT(s1[:], rj[:], s1[:], op.subtract)
                    TS(s2[:].bitcast(i32), s1[:].bitcast(i32), 0x7FFFFFFF, op.bitwise_and)
                    TS(s3[:], nj[:], CC(C_TZ), op.mult, CC(c_t), op.add)
                    TS(s3[:].bitcast(i32), s3[:].bitcast(i32), 0x7FFFFFFF, op.bitwise_and)
                    RF(s1[:], s3[:])
                    TT(o, s2[:], s1[:], op.mult)

                invx, invy = pxs, pys  # dead after mx/my; reuse
                inv_axis(invx[:], nx, rx, C_TX, t3_, t4_, t5_)
                inv_axis(invy[:], ny, ry, C_TY, t6_, t7_, t1_)

                seld, selA = m1, m2  # dead after ps2/pe2; reuse
                V.select(out=seld[:], mask=fmi, on_true=invy[:], on_false=invx[:])
                TS(selA[:], seld[:], CC(C_CA), op.mult, CC(C_CB), op.add)
                TS(inv16[:], selA[:], 0.0, op.max, 1.0, op.min)

            # zero the halo pixels that fall outside the image (edge chunks)
            nc.vector.tensor_tensor(
                out=inv16[:, 320:330], in0=inv16[:, 320:330], in1=hm[:], op=op.mult
            )

            # ---- on-chip 3-row halo build (SBUF->SBUF DMAs, no DRAM trip) ----
            nc.vector.tensor_scalar(
                out=sap(t3i[:], 322 + 1, [[1, 320]]),
                in0=sap(inv16[:], 0, [[1, 320]]),
                scalar1=0.0, scalar2=None, op0=op.bypass,
            )
            # middle tap plane only needs t3i row 1 (no halo DMAs)
            nc.vector.tensor_scalar(
                out=sap(ufi[:], 3 * 320, [[320, 3], [1, 320]]),
                in0=sap(t3i[:], 322, [[1, 3], [1, 320]]),
                scalar1=0.0, scalar2=None, op0=op.bypass,
            )
            # single 127-partition shifts; the nn-boundary rows are then
            # overwritten by the packed halo strips
            nc.sync.dma_start(
                out=sap(t3i[1:128], 1, [[1, 320]]),
                in_=sap(inv16[0:127], 0, [[1, 320]]),
            )
            nc.sync.dma_start(
                out=sap(t3i[0:127], 2 * 322 + 1, [[1, 320]]),
                in_=sap(inv16[1:128], 0, [[1, 320]]),
            )
            for nn in range(NN):
                b0 = nn * 64
                nc.sync.dma_start(
                    out=sap(t3i[b0 : b0 + 1], 1, [[1, 320]]),
                    in_=inv16[b0 : b0 + 64, 320:325],
                )
                nc.sync.dma_start(
                    out=sap(t3i[b0 + 63 : b0 + 64], 2 * 322 + 1, [[1, 320]]),
                    in_=inv16[b0 : b0 + 64, 325:330],
                )
            # top/bottom tap planes once the halo rows are in place
            for dy in (0, 2):
                nc.vector.tensor_scalar(
                    out=sap(ufi[:], 3 * dy * 320, [[320, 3], [1, 320]]),
                    in0=sap(t3i[:], dy * 322, [[1, 3], [1, 320]]),
                    scalar1=0.0, scalar2=None, op0=op.bypass,
                )

            # ---------------- chunk loop ----------------
            with tc.tile_pool(name="work", bufs=2) as cp, tc.tile_pool(
                name="work1", bufs=1
            ) as cp1:
                tails = []

                def emit_tail(t):
                    a32, gc, wc = t
                    dst = dram_ap(
                        out_d, gc * (W * UP) + wc * PL,
                        [[UP * W * UP, 64], [1, PL]],
                    )
                    nc.sync.dma_start(out=dst, in_=a32[:])

                for ci, (wc, gc) in enumerate(CHUNKS):
                    w0 = wc * WC
                    EM = pre[ci] if ci < 3 else emit_head(ci)
                    # e * uf tap products -> planes 9-17 (inv), 18-26 (conf)
                    for tg, uf in ((2, ufc), (1, ufi)):
                        nc.vector.tensor_tensor(
                            out=sap(EM[:], tg * 9 * PL, [[PL, 9], [WC, 4], [1, WC]]),
                            in0=sap(EM[:], 0, [[PL, 9], [WC, 4], [1, WC]]),
                            in1=sap(uf[:], w0, [[320, 9], [0, 4], [1, WC]]),
                            op=op.mult,
                        )
                    # joint 9-tap sum tree over tags (e, e*ufi, e*ufc)
                    t1 = cp1.tile([128, 12, PL], f16, name="t1", tag="t1")
                    nc.vector.tensor_tensor(
                        out=sap(t1[:], 0, [[4 * PL, 3], [PL, 4], [1, PL]]),
                        in0=sap(EM[:], 0, [[9 * PL, 3], [2 * PL, 4], [1, PL]]),
                        in1=sap(EM[:], PL, [[9 * PL, 3], [2 * PL, 4], [1, PL]]),
                        op=op.add,
                    )
                    t2 = cp1.tile([128, 6, PL], f16, name="t2", tag="t2")
                    nc.vector.tensor_tensor(
                        out=sap(t2[:], 0, [[2 * PL, 3], [PL, 2], [1, PL]]),
                        in0=sap(t1[:], 0, [[4 * PL, 3], [2 * PL, 2], [1, PL]]),
                        in1=sap(t1[:], PL, [[4 * PL, 3], [2 * PL, 2], [1, PL]]),
                        op=op.add,
                    )
                    # level 3 writes (s, num_i, num_c) partial sums; the 9th
                    # tap is then folded in by a DMA with compute (dst += src),
                    # which keeps level 4 off the DVE entirely
                    nsic = cp.tile([128, 3, PL], f16, name="nsic", tag="nsic")
                    nc.vector.tensor_tensor(
                        out=sap(nsic[:], 0, [[PL, 3], [1, PL]]),
                        in0=sap(t2[:], 0, [[2 * PL, 3], [1, PL]]),
                        in1=sap(t2[:], PL, [[2 * PL, 3], [1, PL]]),
                        op=op.add,
                    )
                    nc.gpsimd.dma_start(
                        out=sap(nsic[:], 0, [[PL, 3], [1, PL]]),
                        in_=sap(EM[:], 8 * PL, [[9 * PL, 3], [1, PL]]),
                        accum_op=op.add,
                    )
                    # rs = 1/s = exp(-ln(s)) on ACT (s >= 9*min(e) > 0)
                    rs16 = cp.tile([128, PL], f16, name="rs16", tag="rs16")
                    nc.scalar.activation(out=rs16[:], in_=nsic[:, 0, :], func=Act.Ln)
                    nc.scalar.activation(out=rs16[:], in_=rs16[:], func=Act.Exp, scale=-1.0)
                    # weighted averages iu (inv) and cu (conf) in one op
                    iucu = cp.tile([128, 2, PL], f16, name="iucu", tag="iucu")
                    nc.vector.tensor_tensor(
                        out=sap(iucu[:], 0, [[PL, 2], [1, PL]]),
                        in0=sap(nsic[:], PL, [[PL, 2], [1, PL]]),
                        in1=sap(rs16[:], 0, [[0, 2], [1, PL]]),
                        op=op.mult,
                    )
                    # move nn1 halves down to partitions 0-63 for the 2-view fusion
                    iucu2 = cp.tile([64, 2, PL], f16, name="iucu2", tag="iucu2")
                    nc.sync.dma_start(out=iucu2[:], in_=iucu[64:128])
                    lo = slice(0, 64)
                    # fused = (iu0 + iu1*e^dl) / (1 + e^dl), dl = cu1 - cu0
                    dl = cp.tile([64, PL], f16, name="dl", tag="dl")
                    nc.vector.tensor_tensor(
                        out=dl[:], in0=iucu2[:, 1, :], in1=iucu[lo][:, 1, :], op=op.subtract
                    )
                    fb = cp.tile([64, PL], f16, name="fb", tag="fb", bufs=1)
                    nc.scalar.activation(out=fb[:], in_=dl[:], func=Act.Exp)
                    fd = cp.tile([64, PL], f16, name="fd", tag="fd", bufs=1)
                    nc.scalar.activation(out=fd[:], in_=fb[:], func=Act.Ln, bias=1.0)
                    nc.scalar.activation(out=fd[:], in_=fd[:], func=Act.Exp, scale=-1.0)
                    t1f = cp.tile([64, PL], f16, name="t1f", tag="t1f")
                    nc.vector.tensor_tensor(
                        out=t1f[:], in0=iucu2[:, 0, :], in1=fb[:], op=op.mult
                    )
                    nc.vector.tensor_tensor(
                        out=t1f[:], in0=t1f[:], in1=iucu[lo][:, 0, :], op=op.add
                    )
                    nc.vector.tensor_tensor(out=t1f[:], in0=t1f[:], in1=fd[:], op=op.mult)
                    # out = 1/(fused*(dmin-dmax)+dmax) = exp(-ln(affine(fused)))
                    # (w,q2)-interleaved write so the output DMA is contiguous
                    nl = cp.tile([64, PL], f16, name="nl", tag="nl")
                    nc.scalar.activation(
                        out=nl[:], in_=t1f[:], func=Act.Ln,
                        scale=CC(C_DS2, 0, 64), bias=CC(C_DB, 0, 64),
                    )
                    a32 = cp.tile([64, PL], f32, name="a32", tag="a32")
                    nc.scalar.activation(
                        out=sap(a32[:], 0, [[1, UP], [UP, WC]]),
                        in_=sap(nl[:], 0, [[WC, UP], [1, WC]]),
                        func=Act.Exp, scale=-1.0,
                    )
                    tails.append((a32, gc, wc))
                    if ci >= 1:
                        emit_tail(tails[ci - 1])
                emit_tail(tails[-1])

    nc.finalize()
    return nc


def _host_prep(inputs):
    K_ref = np.asarray(inputs["K_ref"], np.float32)
    K_nei = np.asarray(inputs["K_nei"], np.float32)
    R_nei = np.asarray(inputs["R_nei"], np.float32)
    T_nei = np.asarray(inputs["T_nei"], np.float32)
    depth0 = np.asarray(inputs["depth0"], np.float32)
    flow = np.asarray(inputs["flow"], np.float32)
    mask = np.asarray(inputs["mask"], np.float32)
    conf = np.asarray(inputs["conf"], np.float32)
    dmin = float(np.asarray(inputs["depth_min"]).reshape(-1)[0])
    dmax = float(np.asarray(inputs["depth_max"]).reshape(-1)[0])

    # pixel rays per batch (u, v with unit z)
    uv = []
    for b in range(B):
        Ki = np.linalg.inv(K_ref[b, 0, 0].astype(np.float64))
        gx, gy = np.meshgrid(np.arange(W, dtype=np.float64), np.arange(H, dtype=np.float64))
        x = Ki[0, 0] * gx + Ki[0, 1] * gy + Ki[0, 2]
        y = Ki[1, 0] * gx + Ki[1, 1] * gy + Ki[1, 2]
        z = Ki[2, 0] * gx + Ki[2, 1] * gy + Ki[2, 2]
        uv.append((np.float32(x / z), np.float32(y / z)))

    cA = 1.0 / (dmin - dmax)
    cB = -dmax / (dmin - dmax)

    in_maps = []
    for c in range(NCORES):
        b, rc = c // 4, c % 4
        r0 = rc * RPC
        rtop = max(r0 - 1, 0)
        rbot = min(r0 + RPC, H - 1)

        consts = np.zeros((128, NCONST), np.float32)
        for nn in range(NN):
            Kn = K_nei[nn, b, 0, 0].astype(np.float64)
            Rn = R_nei[nn, b, 0, 0].astype(np.float64)
            Tn = T_nei[nn, b, 0, 0].astype(np.float64).reshape(3)
            M = Kn @ Rn
            t = (Kn @ Tn.reshape(3, 1)).reshape(3)
            iK = np.linalg.inv(Kn)
            assert abs(iK[2, 0]) < 1e-12 and abs(iK[2, 1]) < 1e-12 and abs(iK[2, 2] - 1) < 1e-9
            row = np.zeros(NCONST, np.float32)
            row[C_M00:C_M22 + 1] = M.reshape(-1)
            row[C_T0:C_T2 + 1] = t
            row[C_R00:C_R22 + 1] = Rn.reshape(-1)
            row[C_A0:C_A2 + 1] = iK[0] / (1.0 + EPS)
            row[C_B0:C_B2 + 1] = iK[1] / (1.0 + EPS)
            # C_TX/C_TY feed |tz*n + c| as ACT affine bias -> store negated
            row[C_TX], row[C_TY], row[C_TZ] = -Tn[0], -Tn[1], Tn[2]
            row[C_CA], row[C_CB] = cA, cB
            row[C_TEN] = 10.0
            row[C_DS2], row[C_DB] = dmin - dmax, dmax
            consts[nn * 64 : nn * 64 + 64] = row

        u_full, v_full = uv[b]
        d_full = depth0[b, 0]

        pix = np.zeros((128, 4, 330), np.float32)
        for nn in range(NN):
            sl = slice(nn * 64, nn * 64 + 64)
            f_full = flow[nn, b, 0]
            for ch, arr in enumerate((u_full, v_full, d_full, f_full)):
                pix[sl, ch, 0:320] = arr[r0 : r0 + RPC]
                pix[sl, ch, 320:325] = arr[rtop].reshape(64, 5)
                pix[sl, ch, 325:330] = arr[rbot].reshape(64, 5)

        hm = np.ones((128, 10), np.float16)
        if r0 == 0:
            hm[:, 0:5] = 0.0
        if r0 + RPC == H:
            hm[:, 5:10] = 0.0

        confpad = np.zeros((NN, 66, 322), np.float16)
        confpad[:, 1:65, 1:321] = conf[:, b, 0, r0 : r0 + RPC, :]
        if r0 > 0:
            confpad[:, 0, 1:321] = conf[:, b, 0, r0 - 1, :]
        if r0 + RPC < H:
            confpad[:, 65, 1:321] = conf[:, b, 0, r0 + RPC, :]

        # [nn, k, q1, q2, r, wc, w] -> [q1, wc, (nn, r), k, q2, w]
        ms = mask[:, b, :, r0 : r0 + RPC, :].reshape(NN, 9, 4, 4, RPC, 2, WC)
        mask_pk = np.ascontiguousarray(ms.transpose(2, 5, 0, 4, 1, 3, 6)).reshape(
            4, 2, 128, KQ
        ).astype(np.float16)

        in_maps.append(
            {
                "pix": pix,
                "consts": consts,
                "hm": hm,
                "confpad": confpad,
                "maskpk": mask_pk,
            }
        )
    return in_maps


def kernel(**inputs):
    if "nc" not in _cache:
        _cache["nc"] = _build_program()
    nc = _cache["nc"]
    in_maps = _host_prep(inputs)

    from concourse import bass_utils

    res = bass_utils.run_bass_kernel_spmd(nc, in_maps, core_ids=list(range(NCORES)))
    out = np.empty((B, 1, H * UP, W * UP), np.float32)
    for c in range(NCORES):
        b, rc = c // 4, c % 4
        out[b, 0, rc * RPC * UP : (rc + 1) * RPC * UP, :] = res.results[c]["out"]
    return out
